# revision 16
# baseline (speedup 1.0000x reference)
"""Trainium2 Bass kernel for the Brill-Lindquist Christoffel-symbol grid.

Math: the reference reduces to
    psi  = 1 + sum_n m_n / (2 r_n),   m = softplus(pre)
    h    = psi^4
    G_c  = finite-difference gradient of h along grid axis c (2nd order
           central interior, 1st order one-sided edges, spacing DX)
    W_c  = 0.5 * G_c / h
    Gamma^i_{jk} = delta_ij W_k + delta_ik W_j - delta_jk W_i
so the [96,96,96,3,3,3] output is +-W_c scattered over 27 slots per point.

Sharding: axis 0 (12 planes per core x 8 cores). h is analytic in the
inputs, so each core evaluates its slab plus a 1-plane halo directly --
no inter-core exchange. Per core the grid is row-packed: row = a0*96+a1
(1152 rows -> 9 tiles of 128 partitions), free dim = a2 (96); h lives on
an 11-tile extended row window (halo tiles at both ends).

Axis-0/1 derivatives are bf16 matmuls against per-core FD matrices
(edge one-sided differences folded into the matrix entries); the axis-2
derivative is a shifted free-dim subtract scaled by a per-column vector.
A single bf16 copy of h feeds the matmuls: the rel-err budget (2e-2 of
the global absmax) dwarfs the ~4e-3 it costs.

Output assembly exploits Gamma = delta_ij W_k + delta_ik W_j - delta_jk W_i
directly: each delta-term is ONE strided copy covering 9 slots per point
(stride-0 source broadcast over the free index), with the i=j=k slots
written by all three in a fixed order so the last write (+W_i) is the
correct value. Zero slots persist in reused output buffers.

Schedule: the per-chunk h evaluation is emitted interleaved with the
per-tile output pipeline (chunk0, tile0, chunk1, tiles1-3, ...) so the
1.33 MB/tile output DMAs start within a few us and run back-to-back at
the ~358 GB/s HBM write roofline; all input-derived broadcast rows
(masses/2, (z-pz)^2, (x-px)^2+(y-py)^2 column bases, FD column scale)
are marshaled host-side into one [128, 312] block.
"""

import numpy as np

RES = 96
N_CORES = 8
PLANES = RES // N_CORES        # 12
LROWS = PLANES * RES           # 1152 local rows
NT = LROWS // 128              # 9 local 128-row tiles
EXTNT = NT + 2                 # 11 extended tiles (halo)
NROWS_G = RES * RES            # 9216 global rows
S27 = 27
NOB = 4                        # rotating output buffers

# BB broadcast-block column layout ([128, BBW] fp32, replicated rows)
B_MH = 0        # m1/2, m2/2
B_KV = 2        # z-FD column scale [96]
B_CR = 98       # (z - pz1)^2 [96], (z - pz2)^2 [96]
B_AB = 290      # (x-px)^2+(y-py)^2 column bases [11] x 2
BBW = 312

HCHUNKS = [(0, 3), (3, 6), (6, 9), (9, 11)]   # ext-block ranges
# tiles emitted after each chunk (tile t needs ext blocks t..t+2)
CHUNK_TILES = [(0,), (1, 2, 3), (4, 5, 6), (7, 8)]


def _grid_x():
    # Match the reference grid bit-for-bit: jnp.linspace in fp32 on CPU
    # (the reference's softplus cannot compile for the neuron backend, so
    # it necessarily runs on the jax CPU platform).
    import jax
    import jax.numpy as jnp
    MAX_X = 1.0
    DX = np.float32(MAX_X / (RES / 2 - 1))

    def _ls():
        return jnp.linspace(
            DX * (1 - RES / 2), DX * (RES / 2 - 1), RES, dtype=jnp.float32
        )

    try:
        with jax.default_device(jax.devices("cpu")[0]):
            x = np.asarray(_ls())
    except Exception:
        x = np.asarray(_ls())
    return x, float(DX)


def _fd_sources(idx, coeff_c, coeff_e):
    """(offset, coeff) pairs for d/didx with 1st-order one-sided edges."""
    if idx == 0:
        return [(1, coeff_e), (0, -coeff_e)]
    if idx == RES - 1:
        return [(0, coeff_e), (-1, -coeff_e)]
    return [(1, coeff_c), (-1, -coeff_c)]


def _build_dmat(core, DX):
    """[128, 6*3*128] bf16 FD matrices as matmul lhsT ([q, p] = coeff of
    ext-row q in output row p); 0.5 Christoffel factor folded in. All
    values are +-0.25/DX or +-0.5/DX = +-11.75 / +-23.5, exact in bf16.
    Entries: 0 g0(t=0), 1 g0(interior), 2 g0(t=8), 3..5 g1(t%3)."""
    import ml_dtypes
    c0 = 0.5 * (1.0 / (2.0 * np.float64(DX)))
    ce = 0.5 * (1.0 / np.float64(DX))
    out = np.zeros((128, 6 * 3 * 128), np.float64)

    def fill(entry, t, axis):
        for p in range(128):
            gr = core * LROWS + 128 * t + p
            a = (gr // RES) if axis == 0 else (gr % RES)
            step = RES if axis == 0 else 1
            for off, cf in _fd_sources(a, c0, ce):
                g2 = gr + off * step
                e_ = g2 - core * LROWS + 128
                j = e_ // 128 - t
                q = e_ - 128 * (t + j)
                assert 0 <= j <= 2 and 0 <= q < 128, (core, t, p, off)
                out[q, (entry * 3 + j) * 128 + p] = cf

    fill(0, 0, 0)
    fill(1, 1, 0)
    fill(2, NT - 1, 0)
    for v in range(3):
        fill(3 + v, v, 1)
    return out.astype(ml_dtypes.bfloat16)


def _build_static(core, x, DX):
    slab = core * LROWS
    e = np.arange(EXTNT * 128)
    g = np.clip(slab - 128 + e, 0, NROWS_G - 1)   # clamp halo overrun (unused rows)
    xcol = x[g % RES].reshape(EXTNT, 128).T.copy()     # X coordinate (a1)
    ycol = x[g // RES].reshape(EXTNT, 128).T.copy()    # Y coordinate (a0)
    kvec = np.full(RES, 0.25 / DX, np.float64)
    kvec[0] = kvec[-1] = 0.5 / DX
    return {
        "xcol": xcol.astype(np.float64),
        "ycol": ycol.astype(np.float64),
        "kvec": kvec,
        "dmat": _build_dmat(core, DX),
    }


def _build_bb(static, x, pos, mh):
    """Host-marshaled [128, BBW] broadcast block for one core."""
    bb = np.zeros((128, BBW), np.float64)
    bb[:, B_MH:B_MH + 2] = mh[None, :]
    bb[:, B_KV:B_KV + RES] = static["kvec"][None, :]
    for n in range(2):
        dz = x.astype(np.float64) - pos[n, 2]
        bb[:, B_CR + RES * n:B_CR + RES * (n + 1)] = (dz * dz)[None, :]
        ab = (static["xcol"] - pos[n, 0]) ** 2 + (static["ycol"] - pos[n, 1]) ** 2
        bb[:, B_AB + EXTNT * n:B_AB + EXTNT * (n + 1)] = ab
    return np.ascontiguousarray(bb, np.float32)


def _build_program():
    import dataclasses as _dc

    import concourse.bacc as bacc
    import concourse.mybir as mybir
    import concourse.tile as tile

    DT = mybir.dt.float32
    BF = mybir.dt.bfloat16
    AF = mybir.ActivationFunctionType
    OP = mybir.AluOpType

    nc = bacc.Bacc(None, target_bir_lowering=False, debug=False)
    d_dmat = nc.dram_tensor("dmat", [128, 6 * 3 * 128], BF, kind="ExternalInput")
    d_bb = nc.dram_tensor("bb", [128, BBW], DT, kind="ExternalInput")
    d_out = nc.dram_tensor("out", [LROWS, RES * S27], DT, kind="ExternalOutput")

    HW_ = EXTNT * RES             # 1056: free width of the ext h field
    with tile.TileContext(nc) as tc:
        with (
            tc.tile_pool(name="const", bufs=1) as cpool,
            tc.tile_pool(name="work", bufs=3) as wpool,
            tc.tile_pool(name="wout", bufs=3) as wopool,
            tc.tile_pool(name="obuf", bufs=1) as opool,
            tc.tile_pool(name="psum", bufs=2, space="PSUM") as pspool,
        ):
            # BB rides the SWDGE path so it is not queued behind the 4x
            # larger dmat transfer on the HWDGE ring
            BBt = cpool.tile([128, BBW], DT)
            nc.gpsimd.dma_start(BBt[:], d_bb[:])
            dm = cpool.tile([128, 6 * 3 * 128], BF)
            nc.sync.dma_start(dm[:], d_dmat[:])

            H = cpool.tile([128, HW_], DT)
            Hb = cpool.tile([128, HW_], BF)
            otiles = []
            for i in range(NOB):
                O = opool.tile([128, RES * S27], DT, tag=f"ob{i}")
                otiles.append(O)

            def emit_chunk(b0, b1):
                # h = psi^4 on ext blocks [b0, b1); fp32 H + bf16 copy.
                # psi-1 = (mh1*r2 + mh2*r1)/(r1*r2): one reciprocal.
                nb = b1 - b0
                W = nb * RES
                csl = slice(RES * b0, RES * b1)
                rr = []
                for n in range(2):
                    r2 = wpool.tile([128, W], DT, tag="r2")
                    r2v = r2[:].rearrange("p (b z) -> p b z", z=RES)
                    crow = BBt[:, B_CR + RES * n:B_CR + RES * (n + 1)]
                    crow_b = _dc.replace(crow, ap=[crow.ap[0], [0, nb], [1, RES]])
                    absl = BBt[:, B_AB + EXTNT * n + b0:B_AB + EXTNT * n + b1]
                    ab_b = _dc.replace(absl, ap=[absl.ap[0], [1, nb], [0, RES]])
                    nc.gpsimd.tensor_add(r2v[:, :, :], crow_b, ab_b)
                    rn = wpool.tile([128, W], DT, tag=f"rr{n}")
                    nc.scalar.activation(rn[:], r2[:], AF.Sqrt)
                    rr.append(rn)
                v = wpool.tile([128, W], DT, tag="v")
                nc.gpsimd.tensor_mul(v[:], rr[0][:], rr[1][:])
                u2 = wpool.tile([128, W], DT, tag="u2")
                nc.scalar.mul(u2[:], rr[0][:], BBt[:, B_MH + 1:B_MH + 2])
                u = wpool.tile([128, W], DT, tag="u")
                nc.vector.scalar_tensor_tensor(
                    u[:], rr[1][:], BBt[:, B_MH:B_MH + 1], u2[:], OP.mult, OP.add
                )
                vinv = wpool.tile([128, W], DT, tag="vinv")
                nc.vector.reciprocal_approx_fast(vinv[:], v[:])
                psim = wpool.tile([128, W], DT, tag="psim")
                nc.vector.tensor_mul(psim[:], u[:], vinv[:])
                hsq = wpool.tile([128, W], DT, tag="hsq")
                nc.scalar.activation(hsq[:], psim[:], AF.Square, bias=1.0)
                nc.vector.tensor_mul(H[:, csl], hsq[:], hsq[:])
                nc.scalar.activation(Hb[:, csl], hsq[:], AF.Square)

            def emit_tile(t, split):
                O = otiles[t % NOB]
                O3 = O[:].rearrange("p (s z) -> p s z", z=RES)
                if t < NOB:   # one-time zero fill of the 6 zero slots
                    nc.gpsimd.memset(O3[:, 5:8:2, :], 0.0)
                    nc.gpsimd.memset(O3[:, 11:20:4, :], 0.0)
                    nc.gpsimd.memset(O3[:, 21:22, :], 0.0)
                g0e = 0 if t == 0 else (2 if t == NT - 1 else 1)
                g1e = 3 + (t % 3)
                hsl = slice(RES * (t + 1), RES * (t + 2))
                p0 = pspool.tile([128, RES], DT, tag="p0")
                p1 = pspool.tile([128, RES], DT, tag="p1")
                for ge, pp in ((g0e, p0), (g1e, p1)):
                    for j in range(3):
                        lhs = dm[:, (ge * 3 + j) * 128:(ge * 3 + j + 1) * 128]
                        rsl = slice(RES * (t + j), RES * (t + j + 1))
                        nc.tensor.matmul(
                            pp[:], lhs, Hb[:, rsl], start=(j == 0), stop=(j == 2)
                        )

                Hc = H[:, hsl]
                hinv = wopool.tile([128, RES], DT, tag="hinv")
                nc.vector.reciprocal_approx_fast(hinv[:], Hc)
                hz = wopool.tile([128, RES], DT, tag="hz")
                nc.gpsimd.tensor_mul(hz[:], hinv[:], BBt[:, B_KV:B_KV + RES])
                st = wopool.tile([128, RES], DT, tag="st")
                nc.gpsimd.tensor_sub(st[:, 1:95], Hc[:, 2:96], Hc[:, 0:94])
                nc.gpsimd.tensor_sub(st[:, 0:1], Hc[:, 1:2], Hc[:, 0:1])
                nc.gpsimd.tensor_sub(st[:, 95:96], Hc[:, 95:96], Hc[:, 94:95])
                # W planes, slot-major: w3 = [W0 | W1 | W2], 96 each
                w3 = wopool.tile([128, 3 * RES], DT, tag="w3")
                nc.vector.tensor_mul(w3[:, 0:RES], p0[:], hinv[:])
                nc.vector.tensor_mul(w3[:, RES:2 * RES], p1[:], hinv[:])
                nc.vector.tensor_mul(w3[:, 2 * RES:3 * RES], st[:], hz[:])

                # Gamma^i_{jk} = d_ij W_k + d_ik W_j - d_jk W_i: three
                # 9-slot pattern writes (dst element offset 96*slot + z,
                # slots 9i+4j / 10i+3j / 12i+k). The i=j=k slots are
                # written by all three, C -> B -> A; A's +W_i is the
                # correct final value. Each i's slots live in the
                # contiguous third [864*i, 864*(i+1)), so a split tile
                # DMAs each third as soon as its three writes land.
                OW = RES * S27
                for i0, ni in ((0, 1), (1, 1), (2, 1)) if split else ((0, 3),):
                    cd = O[:, 864 * i0:OW]
                    cs = w3[:, RES * i0:3 * RES]
                    c_dst = _dc.replace(cd, ap=[cd.ap[0], [864, ni], [384, 3], [1, RES]])
                    c_src = _dc.replace(cs, ap=[cs.ap[0], [96, ni], [0, 3], [1, RES]])
                    nc.vector.tensor_scalar_mul(c_dst, c_src, -1.0)   # -d_jk W_i
                    bd = O[:, 960 * i0:OW]
                    b_dst = _dc.replace(bd, ap=[bd.ap[0], [288, 3], [960, ni], [1, RES]])
                    b_src = _dc.replace(w3[:], ap=[w3[:].ap[0], [96, 3], [0, ni], [1, RES]])
                    nc.gpsimd.tensor_copy(b_dst, b_src)               # d_ik W_j
                    ad = O[:, 1152 * i0:OW]
                    a_dst = _dc.replace(ad, ap=[ad.ap[0], [96, 3], [1152, ni], [1, RES]])
                    a_src = _dc.replace(w3[:], ap=[w3[:].ap[0], [96, 3], [0, ni], [1, RES]])
                    nc.scalar.copy(a_dst, a_src)                      # d_ij W_k
                    nc.sync.dma_start(
                        d_out[128 * t:128 * (t + 1), 864 * i0:864 * (i0 + ni)],
                        O[:, 864 * i0:864 * (i0 + ni)],
                    )

            # first and last tiles stream out in thirds: pulls the first
            # output DMA earlier and drains the tail sooner
            for (b0, b1), tiles in zip(HCHUNKS, CHUNK_TILES):
                emit_chunk(b0, b1)
                for t in tiles:
                    emit_tile(t, split=t in (0, NT - 1))

    nc.finalize()
    return nc


_CACHE = {}


def _get_setup():
    if "nc" not in _CACHE:
        x, DX = _grid_x()
        _CACHE["x"] = x
        _CACHE["static"] = [_build_static(c, x, DX) for c in range(N_CORES)]
        _CACHE["nc"] = _build_program()
    return _CACHE["nc"], _CACHE["static"], _CACHE["x"]


def kernel(BH_positions, BH_masses_presoftplus):
    from concourse.bass_utils import run_bass_kernel_spmd

    nc, static, x = _get_setup()
    pos = np.asarray(BH_positions, np.float64).reshape(2, 3)
    # softplus of the two mass parameters (log1p(exp(x)) in fp32, as jax.nn.softplus)
    pre = np.asarray(BH_masses_presoftplus, np.float32)
    mh = 0.5 * np.log1p(np.exp(pre)).astype(np.float32).astype(np.float64)
    in_maps = [
        {"dmat": static[c]["dmat"], "bb": _build_bb(static[c], x, pos, mh)}
        for c in range(N_CORES)
    ]
    res = run_bass_kernel_spmd(nc, in_maps, list(range(N_CORES)))
    # device tiles are slot-major ([row, s, z]); unshard transposes back
    parts = [
        res.results[c]["out"]
        .reshape(PLANES, RES, S27, RES)
        .transpose(0, 1, 3, 2)
        .reshape(PLANES, RES, RES, 3, 3, 3)
        for c in range(N_CORES)
    ]
    return np.ascontiguousarray(np.concatenate(parts, axis=0))


# revision 18
# speedup vs baseline: 1.2084x; 1.2084x over previous
"""Trainium2 Bass kernel for the Brill-Lindquist Christoffel-symbol grid.

Math: the reference reduces to
    psi  = 1 + sum_n m_n / (2 r_n),   m = softplus(pre)
    h    = psi^4
    G_c  = finite-difference gradient of h along grid axis c (2nd order
           central interior, 1st order one-sided edges, spacing DX)
    W_c  = 0.5 * G_c / h
    Gamma^i_{jk} = delta_ij W_k + delta_ik W_j - delta_jk W_i
so the [96,96,96,3,3,3] output is +-W_c scattered over 27 slots per point.

Sharding: axis 0 (12 planes per core x 8 cores). h is analytic in the
inputs, so each core evaluates its slab plus a 1-plane halo directly --
no inter-core exchange. Per core the grid is row-packed: row = a0*96+a1
(1152 rows -> 9 tiles of 128 partitions), free dim = a2 (96); h lives on
an 11-tile extended row window (halo tiles at both ends).

Axis-0/1 derivatives are bf16 matmuls against per-core FD matrices
(edge one-sided differences folded into the matrix entries); the axis-2
derivative is a shifted free-dim subtract scaled by a per-column vector.
A single bf16 copy of h feeds the matmuls: the rel-err budget (2e-2 of
the global absmax) dwarfs the ~4e-3 it costs.

Output assembly exploits Gamma = delta_ij W_k + delta_ik W_j - delta_jk W_i
directly: each delta-term is ONE strided copy covering 9 slots per point
(stride-0 source broadcast over the free index), with the i=j=k slots
written by all three in a fixed order so the last write (+W_i) is the
correct value. Zero slots persist in reused output buffers.

Schedule: the per-chunk h evaluation is emitted interleaved with the
per-tile output pipeline (chunk0, tile0, chunk1, tiles1-3, ...) so the
1.33 MB/tile output DMAs start within a few us and run back-to-back at
the ~358 GB/s HBM write roofline; all input-derived broadcast rows
(masses/2, (z-pz)^2, (x-px)^2+(y-py)^2 column bases, FD column scale)
are marshaled host-side into one [128, 312] block.
"""

import numpy as np

RES = 96
N_CORES = 8
PLANES = RES // N_CORES        # 12
LROWS = PLANES * RES           # 1152 local rows
NT = LROWS // 128              # 9 local 128-row tiles
EXTNT = NT + 2                 # 11 extended tiles (halo)
NROWS_G = RES * RES            # 9216 global rows
S27 = 27
NOB = 4                        # rotating output buffers

# BB broadcast-block column layout ([128, BBW] fp32, replicated rows)
B_MH = 0        # m1/2, m2/2
B_KV = 2        # z-FD column scale [96]
B_CR = 98       # (z - pz1)^2 [96], (z - pz2)^2 [96]
B_AB = 290      # (x-px)^2+(y-py)^2 column bases [11] x 2
BBW = 312

HCHUNKS = [(0, 3), (3, 6), (6, 9), (9, 11)]   # ext-block ranges
# tiles emitted after each chunk (tile t needs ext blocks t..t+2)
CHUNK_TILES = [(0,), (1, 2, 3), (4, 5, 6), (7, 8)]


def _grid_x():
    # Match the reference grid bit-for-bit: jnp.linspace in fp32 on CPU
    # (the reference's softplus cannot compile for the neuron backend, so
    # it necessarily runs on the jax CPU platform).
    import jax
    import jax.numpy as jnp
    MAX_X = 1.0
    DX = np.float32(MAX_X / (RES / 2 - 1))

    def _ls():
        return jnp.linspace(
            DX * (1 - RES / 2), DX * (RES / 2 - 1), RES, dtype=jnp.float32
        )

    try:
        with jax.default_device(jax.devices("cpu")[0]):
            x = np.asarray(_ls())
    except Exception:
        x = np.asarray(_ls())
    return x, float(DX)


def _fd_sources(idx, coeff_c, coeff_e):
    """(offset, coeff) pairs for d/didx with 1st-order one-sided edges."""
    if idx == 0:
        return [(1, coeff_e), (0, -coeff_e)]
    if idx == RES - 1:
        return [(0, coeff_e), (-1, -coeff_e)]
    return [(1, coeff_c), (-1, -coeff_c)]


def _build_dmat(core, DX):
    """[128, 6*3*128] bf16 FD matrices as matmul lhsT ([q, p] = coeff of
    ext-row q in output row p); 0.5 Christoffel factor folded in. All
    values are +-0.25/DX or +-0.5/DX = +-11.75 / +-23.5, exact in bf16.
    Entries: 0 g0(t=0), 1 g0(interior), 2 g0(t=8), 3..5 g1(t%3)."""
    import ml_dtypes
    c0 = 0.5 * (1.0 / (2.0 * np.float64(DX)))
    ce = 0.5 * (1.0 / np.float64(DX))
    out = np.zeros((128, 6 * 3 * 128), np.float64)

    def fill(entry, t, axis):
        for p in range(128):
            gr = core * LROWS + 128 * t + p
            a = (gr // RES) if axis == 0 else (gr % RES)
            step = RES if axis == 0 else 1
            for off, cf in _fd_sources(a, c0, ce):
                g2 = gr + off * step
                e_ = g2 - core * LROWS + 128
                j = e_ // 128 - t
                q = e_ - 128 * (t + j)
                assert 0 <= j <= 2 and 0 <= q < 128, (core, t, p, off)
                out[q, (entry * 3 + j) * 128 + p] = cf

    fill(0, 0, 0)
    fill(1, 1, 0)
    fill(2, NT - 1, 0)
    for v in range(3):
        fill(3 + v, v, 1)
    return out.astype(ml_dtypes.bfloat16)


def _build_static(core, x, DX):
    slab = core * LROWS
    e = np.arange(EXTNT * 128)
    g = np.clip(slab - 128 + e, 0, NROWS_G - 1)   # clamp halo overrun (unused rows)
    xcol = x[g % RES].reshape(EXTNT, 128).T.copy()     # X coordinate (a1)
    ycol = x[g // RES].reshape(EXTNT, 128).T.copy()    # Y coordinate (a0)
    kvec = np.full(RES, 0.25 / DX, np.float64)
    kvec[0] = kvec[-1] = 0.5 / DX
    return {
        "xcol": xcol.astype(np.float64),
        "ycol": ycol.astype(np.float64),
        "kvec": kvec,
        "dmat": _build_dmat(core, DX),
    }


def _build_bb(static, x, pos, mh):
    """Host-marshaled [128, BBW] broadcast block for one core."""
    bb = np.zeros((128, BBW), np.float64)
    bb[:, B_MH:B_MH + 2] = mh[None, :]
    bb[:, B_KV:B_KV + RES] = static["kvec"][None, :]
    for n in range(2):
        dz = x.astype(np.float64) - pos[n, 2]
        bb[:, B_CR + RES * n:B_CR + RES * (n + 1)] = (dz * dz)[None, :]
        ab = (static["xcol"] - pos[n, 0]) ** 2 + (static["ycol"] - pos[n, 1]) ** 2
        bb[:, B_AB + EXTNT * n:B_AB + EXTNT * (n + 1)] = ab
    return np.ascontiguousarray(bb, np.float32)


def _build_program():
    import dataclasses as _dc

    import concourse.bacc as bacc
    import concourse.mybir as mybir
    import concourse.tile as tile

    DT = mybir.dt.float32
    BF = mybir.dt.bfloat16
    AF = mybir.ActivationFunctionType
    OP = mybir.AluOpType

    nc = bacc.Bacc(None, target_bir_lowering=False, debug=False)
    d_dmat = nc.dram_tensor("dmat", [128, 6 * 3 * 128], BF, kind="ExternalInput")
    d_bb = nc.dram_tensor("bb", [128, BBW], DT, kind="ExternalInput")
    d_out = nc.dram_tensor("out", [LROWS, RES * S27], DT, kind="ExternalOutput")

    HW_ = EXTNT * RES             # 1056: free width of the ext h field
    with tile.TileContext(nc) as tc:
        with (
            tc.tile_pool(name="const", bufs=1) as cpool,
            tc.tile_pool(name="work", bufs=3) as wpool,
            tc.tile_pool(name="wout", bufs=3) as wopool,
            tc.tile_pool(name="obuf", bufs=1) as opool,
            tc.tile_pool(name="psum", bufs=2, space="PSUM") as pspool,
        ):
            # BB rides the SWDGE path so it is not queued behind the 4x
            # larger dmat transfer on the HWDGE ring
            BBt = cpool.tile([128, BBW], DT)
            nc.gpsimd.dma_start(BBt[:], d_bb[:])
            dm = cpool.tile([128, 6 * 3 * 128], BF)
            nc.sync.dma_start(dm[:], d_dmat[:])

            H = cpool.tile([128, HW_], DT)
            Hb = cpool.tile([128, HW_], BF)
            otiles = []
            for i in range(NOB):
                O = opool.tile([128, RES * S27], DT, tag=f"ob{i}")
                otiles.append(O)

            def emit_chunk(b0, b1):
                # h = psi^4 on ext blocks [b0, b1); fp32 H + bf16 copy.
                # psi-1 = mh1/r1 + mh2/r2 via the rsqrt activation table
                # (plenty of rel-err budget for its limited precision).
                nb = b1 - b0
                W = nb * RES
                csl = slice(RES * b0, RES * b1)
                qq = []
                for n in range(2):
                    r2 = wpool.tile([128, W], DT, tag="r2")
                    r2v = r2[:].rearrange("p (b z) -> p b z", z=RES)
                    crow = BBt[:, B_CR + RES * n:B_CR + RES * (n + 1)]
                    crow_b = _dc.replace(crow, ap=[crow.ap[0], [0, nb], [1, RES]])
                    absl = BBt[:, B_AB + EXTNT * n + b0:B_AB + EXTNT * n + b1]
                    ab_b = _dc.replace(absl, ap=[absl.ap[0], [1, nb], [0, RES]])
                    nc.gpsimd.tensor_add(r2v[:, :, :], crow_b, ab_b)
                    qn = wpool.tile([128, W], DT, tag=f"q{n}")
                    nc.scalar.activation(qn[:], r2[:], AF.Abs_reciprocal_sqrt)
                    qq.append(qn)
                u2 = wpool.tile([128, W], DT, tag="u2")
                nc.vector.tensor_scalar_mul(u2[:], qq[1][:], BBt[:, B_MH + 1:B_MH + 2])
                psim = wpool.tile([128, W], DT, tag="psim")
                nc.vector.scalar_tensor_tensor(
                    psim[:], qq[0][:], BBt[:, B_MH:B_MH + 1], u2[:], OP.mult, OP.add
                )
                hsq = wpool.tile([128, W], DT, tag="hsq")
                nc.scalar.activation(hsq[:], psim[:], AF.Square, bias=1.0)
                nc.vector.tensor_mul(H[:, csl], hsq[:], hsq[:])
                nc.scalar.activation(Hb[:, csl], hsq[:], AF.Square)

            def emit_tile(t, split):
                O = otiles[t % NOB]
                O3 = O[:].rearrange("p (s z) -> p s z", z=RES)
                if t < NOB:   # one-time zero fill of the 6 zero slots
                    nc.gpsimd.memset(O3[:, 5:8:2, :], 0.0)
                    nc.gpsimd.memset(O3[:, 11:20:4, :], 0.0)
                    nc.gpsimd.memset(O3[:, 21:22, :], 0.0)
                g0e = 0 if t == 0 else (2 if t == NT - 1 else 1)
                g1e = 3 + (t % 3)
                hsl = slice(RES * (t + 1), RES * (t + 2))
                p0 = pspool.tile([128, RES], DT, tag="p0")
                p1 = pspool.tile([128, RES], DT, tag="p1")
                for ge, pp in ((g0e, p0), (g1e, p1)):
                    for j in range(3):
                        lhs = dm[:, (ge * 3 + j) * 128:(ge * 3 + j + 1) * 128]
                        rsl = slice(RES * (t + j), RES * (t + j + 1))
                        nc.tensor.matmul(
                            pp[:], lhs, Hb[:, rsl], start=(j == 0), stop=(j == 2)
                        )

                Hc = H[:, hsl]
                hinv = wopool.tile([128, RES], DT, tag="hinv")
                nc.vector.reciprocal_approx_fast(hinv[:], Hc)
                hz = wopool.tile([128, RES], DT, tag="hz")
                nc.gpsimd.tensor_mul(hz[:], hinv[:], BBt[:, B_KV:B_KV + RES])
                st = wopool.tile([128, RES], DT, tag="st")
                nc.gpsimd.tensor_sub(st[:, 1:95], Hc[:, 2:96], Hc[:, 0:94])
                nc.gpsimd.tensor_sub(st[:, 0:1], Hc[:, 1:2], Hc[:, 0:1])
                nc.gpsimd.tensor_sub(st[:, 95:96], Hc[:, 95:96], Hc[:, 94:95])
                # W planes, slot-major: w3 = [W0 | W1 | W2], 96 each
                w3 = wopool.tile([128, 3 * RES], DT, tag="w3")
                nc.vector.tensor_mul(w3[:, 0:RES], p0[:], hinv[:])
                nc.vector.tensor_mul(w3[:, RES:2 * RES], p1[:], hinv[:])
                nc.vector.tensor_mul(w3[:, 2 * RES:3 * RES], st[:], hz[:])

                # Gamma^i_{jk} = d_ij W_k + d_ik W_j - d_jk W_i: three
                # 9-slot pattern writes (dst element offset 96*slot + z,
                # slots 9i+4j / 10i+3j / 12i+k). The i=j=k slots are
                # written by all three, C -> B -> A; A's +W_i is the
                # correct final value. Each i's slots live in the
                # contiguous third [864*i, 864*(i+1)), so a split tile
                # DMAs each third as soon as its three writes land.
                OW = RES * S27
                for i0, ni in ((0, 1), (1, 1), (2, 1)) if split else ((0, 3),):
                    cd = O[:, 864 * i0:OW]
                    cs = w3[:, RES * i0:3 * RES]
                    c_dst = _dc.replace(cd, ap=[cd.ap[0], [864, ni], [384, 3], [1, RES]])
                    c_src = _dc.replace(cs, ap=[cs.ap[0], [96, ni], [0, 3], [1, RES]])
                    nc.vector.tensor_scalar_mul(c_dst, c_src, -1.0)   # -d_jk W_i
                    bd = O[:, 960 * i0:OW]
                    b_dst = _dc.replace(bd, ap=[bd.ap[0], [288, 3], [960, ni], [1, RES]])
                    b_src = _dc.replace(w3[:], ap=[w3[:].ap[0], [96, 3], [0, ni], [1, RES]])
                    nc.scalar.copy(b_dst, b_src)                      # d_ik W_j
                    ad = O[:, 1152 * i0:OW]
                    a_dst = _dc.replace(ad, ap=[ad.ap[0], [96, 3], [1152, ni], [1, RES]])
                    a_src = _dc.replace(w3[:], ap=[w3[:].ap[0], [96, 3], [0, ni], [1, RES]])
                    nc.scalar.copy(a_dst, a_src)                      # d_ij W_k
                    nc.sync.dma_start(
                        d_out[128 * t:128 * (t + 1), 864 * i0:864 * (i0 + ni)],
                        O[:, 864 * i0:864 * (i0 + ni)],
                    )

            # first and last tiles stream out in thirds: pulls the first
            # output DMA earlier and drains the tail sooner
            for (b0, b1), tiles in zip(HCHUNKS, CHUNK_TILES):
                emit_chunk(b0, b1)
                for t in tiles:
                    emit_tile(t, split=t in (0, NT - 1))

    nc.finalize()
    return nc


_CACHE = {}


def _get_setup():
    if "nc" not in _CACHE:
        x, DX = _grid_x()
        _CACHE["x"] = x
        _CACHE["static"] = [_build_static(c, x, DX) for c in range(N_CORES)]
        _CACHE["nc"] = _build_program()
    return _CACHE["nc"], _CACHE["static"], _CACHE["x"]


def kernel(BH_positions, BH_masses_presoftplus):
    from concourse.bass_utils import run_bass_kernel_spmd

    nc, static, x = _get_setup()
    pos = np.asarray(BH_positions, np.float64).reshape(2, 3)
    # softplus of the two mass parameters (log1p(exp(x)) in fp32, as jax.nn.softplus)
    pre = np.asarray(BH_masses_presoftplus, np.float32)
    mh = 0.5 * np.log1p(np.exp(pre)).astype(np.float32).astype(np.float64)
    in_maps = [
        {"dmat": static[c]["dmat"], "bb": _build_bb(static[c], x, pos, mh)}
        for c in range(N_CORES)
    ]
    res = run_bass_kernel_spmd(nc, in_maps, list(range(N_CORES)))
    # device tiles are slot-major ([row, s, z]); unshard transposes back
    parts = [
        res.results[c]["out"]
        .reshape(PLANES, RES, S27, RES)
        .transpose(0, 1, 3, 2)
        .reshape(PLANES, RES, RES, 3, 3, 3)
        for c in range(N_CORES)
    ]
    return np.ascontiguousarray(np.concatenate(parts, axis=0))


# revision 20
# speedup vs baseline: 1.3619x; 1.1270x over previous
"""Trainium2 Bass kernel for the Brill-Lindquist Christoffel-symbol grid.

Math: the reference reduces to
    psi  = 1 + sum_n m_n / (2 r_n),   m = softplus(pre)
    h    = psi^4
    G_c  = finite-difference gradient of h along grid axis c (2nd order
           central interior, 1st order one-sided edges, spacing DX)
    W_c  = 0.5 * G_c / h
    Gamma^i_{jk} = delta_ij W_k + delta_ik W_j - delta_jk W_i
so the [96,96,96,3,3,3] output is +-W_c scattered over 27 slots per point.

Sharding: axis 0 (12 planes per core x 8 cores). h is analytic in the
inputs, so each core evaluates its slab plus a 1-plane halo directly --
no inter-core exchange. Per core the grid is row-packed: row = a0*96+a1
(1152 rows -> 9 tiles of 128 partitions), free dim = a2 (96); h lives on
an 11-tile extended row window (halo tiles at both ends).

Axis-0/1 derivatives are bf16 matmuls against per-core FD matrices
(edge one-sided differences folded into the matrix entries); the axis-2
derivative is a shifted free-dim subtract scaled by a per-column vector.
A single bf16 copy of h feeds the matmuls: the rel-err budget (2e-2 of
the global absmax) dwarfs the ~4e-3 it costs.

Output assembly exploits Gamma = delta_ij W_k + delta_ik W_j - delta_jk W_i
directly: each delta-term is ONE strided copy covering 9 slots per point
(stride-0 source broadcast over the free index), with the i=j=k slots
written by all three in a fixed order so the last write (+W_i) is the
correct value. Zero slots persist in reused output buffers.

Schedule: the per-chunk h evaluation is emitted interleaved with the
per-tile output pipeline (chunk0, tile0, chunk1, tiles1-3, ...) so the
1.33 MB/tile output DMAs start within a few us and run back-to-back at
the ~358 GB/s HBM write roofline; all input-derived broadcast rows
(masses/2, (z-pz)^2, (x-px)^2+(y-py)^2 column bases, FD column scale)
are marshaled host-side into one [128, 312] block.
"""

import numpy as np

RES = 96
N_CORES = 8
PLANES = RES // N_CORES        # 12
LROWS = PLANES * RES           # 1152 local rows
NT = LROWS // 128              # 9 local 128-row tiles
EXTNT = NT + 2                 # 11 extended tiles (halo)
NROWS_G = RES * RES            # 9216 global rows
S27 = 27
NOB = 4                        # rotating output buffers

# BB broadcast-block column layout ([128, BBW] fp32, replicated rows)
B_MH = 0        # m1/2, m2/2
B_KV = 2        # z-FD column scale [96]
B_CR = 98       # (z - pz1)^2 [96], (z - pz2)^2 [96]
B_AB = 290      # (x-px)^2+(y-py)^2 column bases [11] x 2
BBW = 312

HCHUNKS = [(0, 3), (3, 6), (6, 9), (9, 11)]   # ext-block ranges
# tiles emitted after each chunk (tile t needs ext blocks t..t+2)
CHUNK_TILES = [(0,), (1, 2, 3), (4, 5, 6), (7, 8)]


def _grid_x():
    # Match the reference grid bit-for-bit: jnp.linspace in fp32 on CPU
    # (the reference's softplus cannot compile for the neuron backend, so
    # it necessarily runs on the jax CPU platform).
    import jax
    import jax.numpy as jnp
    MAX_X = 1.0
    DX = np.float32(MAX_X / (RES / 2 - 1))

    def _ls():
        return jnp.linspace(
            DX * (1 - RES / 2), DX * (RES / 2 - 1), RES, dtype=jnp.float32
        )

    try:
        with jax.default_device(jax.devices("cpu")[0]):
            x = np.asarray(_ls())
    except Exception:
        x = np.asarray(_ls())
    return x, float(DX)


def _fd_sources(idx, coeff_c, coeff_e):
    """(offset, coeff) pairs for d/didx with 1st-order one-sided edges."""
    if idx == 0:
        return [(1, coeff_e), (0, -coeff_e)]
    if idx == RES - 1:
        return [(0, coeff_e), (-1, -coeff_e)]
    return [(1, coeff_c), (-1, -coeff_c)]


def _build_dmat(core, DX):
    """[128, 6*3*128] bf16 FD matrices as matmul lhsT ([q, p] = coeff of
    ext-row q in output row p); 0.5 Christoffel factor folded in. All
    values are +-0.25/DX or +-0.5/DX = +-11.75 / +-23.5, exact in bf16.
    Entries: 0 g0(t=0), 1 g0(interior), 2 g0(t=8), 3..5 g1(t%3)."""
    import ml_dtypes
    c0 = 0.5 * (1.0 / (2.0 * np.float64(DX)))
    ce = 0.5 * (1.0 / np.float64(DX))
    out = np.zeros((128, 6 * 3 * 128), np.float64)

    def fill(entry, t, axis):
        for p in range(128):
            gr = core * LROWS + 128 * t + p
            a = (gr // RES) if axis == 0 else (gr % RES)
            step = RES if axis == 0 else 1
            for off, cf in _fd_sources(a, c0, ce):
                g2 = gr + off * step
                e_ = g2 - core * LROWS + 128
                j = e_ // 128 - t
                q = e_ - 128 * (t + j)
                assert 0 <= j <= 2 and 0 <= q < 128, (core, t, p, off)
                out[q, (entry * 3 + j) * 128 + p] = cf

    fill(0, 0, 0)
    fill(1, 1, 0)
    fill(2, NT - 1, 0)
    for v in range(3):
        fill(3 + v, v, 1)
    return out.astype(ml_dtypes.bfloat16)


def _build_static(core, x, DX):
    slab = core * LROWS
    e = np.arange(EXTNT * 128)
    g = np.clip(slab - 128 + e, 0, NROWS_G - 1)   # clamp halo overrun (unused rows)
    xcol = x[g % RES].reshape(EXTNT, 128).T.copy()     # X coordinate (a1)
    ycol = x[g // RES].reshape(EXTNT, 128).T.copy()    # Y coordinate (a0)
    kvec = np.full(RES, 0.25 / DX, np.float64)
    kvec[0] = kvec[-1] = 0.5 / DX
    return {
        "xcol": xcol.astype(np.float64),
        "ycol": ycol.astype(np.float64),
        "kvec": kvec,
        "dmat": _build_dmat(core, DX),
    }


def _build_bb(static, x, pos, mh):
    """Host-marshaled [128, BBW] broadcast block for one core."""
    bb = np.zeros((128, BBW), np.float64)
    bb[:, B_MH:B_MH + 2] = mh[None, :]
    bb[:, B_KV:B_KV + RES] = static["kvec"][None, :]
    for n in range(2):
        dz = x.astype(np.float64) - pos[n, 2]
        bb[:, B_CR + RES * n:B_CR + RES * (n + 1)] = (dz * dz)[None, :]
        ab = (static["xcol"] - pos[n, 0]) ** 2 + (static["ycol"] - pos[n, 1]) ** 2
        bb[:, B_AB + EXTNT * n:B_AB + EXTNT * (n + 1)] = ab
    return np.ascontiguousarray(bb, np.float32)


def _build_program():
    import dataclasses as _dc

    import concourse.bacc as bacc
    import concourse.mybir as mybir
    import concourse.tile as tile

    DT = mybir.dt.float32
    BF = mybir.dt.bfloat16
    AF = mybir.ActivationFunctionType
    OP = mybir.AluOpType

    nc = bacc.Bacc(None, target_bir_lowering=False, debug=False)
    d_dmat = nc.dram_tensor("dmat", [128, 6 * 3 * 128], BF, kind="ExternalInput")
    d_bb = nc.dram_tensor("bb", [128, BBW], DT, kind="ExternalInput")
    d_out = nc.dram_tensor("out", [LROWS, RES * S27], DT, kind="ExternalOutput")

    HW_ = EXTNT * RES             # 1056: free width of the ext h field
    with tile.TileContext(nc) as tc:
        with (
            tc.tile_pool(name="const", bufs=1) as cpool,
            tc.tile_pool(name="work", bufs=3) as wpool,
            tc.tile_pool(name="wout", bufs=3) as wopool,
            tc.tile_pool(name="obuf", bufs=1) as opool,
            tc.tile_pool(name="psum", bufs=2, space="PSUM") as pspool,
        ):
            # BB gates the whole h pipeline: give it the sync HWDGE ring
            # to itself and put the 4x larger dmat (not needed until the
            # first matmul) on the scalar-engine HWDGE ring
            BBt = cpool.tile([128, BBW], DT)
            nc.sync.dma_start(BBt[:], d_bb[:])
            dm = cpool.tile([128, 6 * 3 * 128], BF)
            nc.scalar.dma_start(dm[:], d_dmat[:])

            H = cpool.tile([128, HW_], DT)
            Hb = cpool.tile([128, HW_], BF)
            otiles = []
            for i in range(NOB):
                O = opool.tile([128, RES * S27], DT, tag=f"ob{i}")
                otiles.append(O)

            def emit_chunk(b0, b1):
                # h = psi^4 on ext blocks [b0, b1); fp32 H + bf16 copy.
                # psi-1 = mh1/r1 + mh2/r2 via the rsqrt activation table
                # (plenty of rel-err budget for its limited precision).
                nb = b1 - b0
                W = nb * RES
                csl = slice(RES * b0, RES * b1)
                qq = []
                for n in range(2):
                    r2 = wpool.tile([128, W], DT, tag="r2")
                    r2v = r2[:].rearrange("p (b z) -> p b z", z=RES)
                    crow = BBt[:, B_CR + RES * n:B_CR + RES * (n + 1)]
                    crow_b = _dc.replace(crow, ap=[crow.ap[0], [0, nb], [1, RES]])
                    absl = BBt[:, B_AB + EXTNT * n + b0:B_AB + EXTNT * n + b1]
                    ab_b = _dc.replace(absl, ap=[absl.ap[0], [1, nb], [0, RES]])
                    nc.gpsimd.tensor_add(r2v[:, :, :], crow_b, ab_b)
                    qn = wpool.tile([128, W], DT, tag=f"q{n}")
                    nc.scalar.activation(qn[:], r2[:], AF.Abs_reciprocal_sqrt)
                    qq.append(qn)
                u2 = wpool.tile([128, W], DT, tag="u2")
                nc.vector.tensor_scalar_mul(u2[:], qq[1][:], BBt[:, B_MH + 1:B_MH + 2])
                psim = wpool.tile([128, W], DT, tag="psim")
                nc.vector.scalar_tensor_tensor(
                    psim[:], qq[0][:], BBt[:, B_MH:B_MH + 1], u2[:], OP.mult, OP.add
                )
                hsq = wpool.tile([128, W], DT, tag="hsq")
                nc.scalar.activation(hsq[:], psim[:], AF.Square, bias=1.0)
                nc.vector.tensor_mul(H[:, csl], hsq[:], hsq[:])
                nc.scalar.activation(Hb[:, csl], hsq[:], AF.Square)

            def emit_tile(t, split):
                O = otiles[t % NOB]
                O3 = O[:].rearrange("p (s z) -> p s z", z=RES)
                if t < NOB:   # one-time zero fill of the 6 zero slots
                    nc.gpsimd.memset(O3[:, 5:8:2, :], 0.0)
                    nc.gpsimd.memset(O3[:, 11:20:4, :], 0.0)
                    nc.gpsimd.memset(O3[:, 21:22, :], 0.0)
                g0e = 0 if t == 0 else (2 if t == NT - 1 else 1)
                g1e = 3 + (t % 3)
                hsl = slice(RES * (t + 1), RES * (t + 2))
                p0 = pspool.tile([128, RES], DT, tag="p0")
                p1 = pspool.tile([128, RES], DT, tag="p1")
                for ge, pp in ((g0e, p0), (g1e, p1)):
                    for j in range(3):
                        lhs = dm[:, (ge * 3 + j) * 128:(ge * 3 + j + 1) * 128]
                        rsl = slice(RES * (t + j), RES * (t + j + 1))
                        nc.tensor.matmul(
                            pp[:], lhs, Hb[:, rsl], start=(j == 0), stop=(j == 2)
                        )

                Hc = H[:, hsl]
                hinv = wopool.tile([128, RES], DT, tag="hinv")
                nc.vector.reciprocal_approx_fast(hinv[:], Hc)
                hz = wopool.tile([128, RES], DT, tag="hz")
                nc.gpsimd.tensor_mul(hz[:], hinv[:], BBt[:, B_KV:B_KV + RES])
                st = wopool.tile([128, RES], DT, tag="st")
                nc.gpsimd.tensor_sub(st[:, 1:95], Hc[:, 2:96], Hc[:, 0:94])
                nc.gpsimd.tensor_sub(st[:, 0:1], Hc[:, 1:2], Hc[:, 0:1])
                nc.gpsimd.tensor_sub(st[:, 95:96], Hc[:, 95:96], Hc[:, 94:95])
                # W planes, slot-major: w3 = [W0 | W1 | W2], 96 each
                w3 = wopool.tile([128, 3 * RES], DT, tag="w3")
                nc.vector.tensor_mul(w3[:, 0:RES], p0[:], hinv[:])
                nc.vector.tensor_mul(w3[:, RES:2 * RES], p1[:], hinv[:])
                nc.vector.tensor_mul(w3[:, 2 * RES:3 * RES], st[:], hz[:])

                # Gamma^i_{jk} = d_ij W_k + d_ik W_j - d_jk W_i: three
                # 9-slot pattern writes (dst element offset 96*slot + z,
                # slots 9i+4j / 10i+3j / 12i+k). The i=j=k slots are
                # written by all three, C -> B -> A; A's +W_i is the
                # correct final value. Each i's slots live in the
                # contiguous third [864*i, 864*(i+1)), so a split tile
                # DMAs each third as soon as its three writes land.
                OW = RES * S27
                for i0, ni in ((0, 1), (1, 1), (2, 1)) if split else ((0, 3),):
                    cd = O[:, 864 * i0:OW]
                    cs = w3[:, RES * i0:3 * RES]
                    c_dst = _dc.replace(cd, ap=[cd.ap[0], [864, ni], [384, 3], [1, RES]])
                    c_src = _dc.replace(cs, ap=[cs.ap[0], [96, ni], [0, 3], [1, RES]])
                    nc.vector.tensor_scalar_mul(c_dst, c_src, -1.0)   # -d_jk W_i
                    bd = O[:, 960 * i0:OW]
                    b_dst = _dc.replace(bd, ap=[bd.ap[0], [288, 3], [960, ni], [1, RES]])
                    b_src = _dc.replace(w3[:], ap=[w3[:].ap[0], [96, 3], [0, ni], [1, RES]])
                    nc.scalar.copy(b_dst, b_src)                      # d_ik W_j
                    ad = O[:, 1152 * i0:OW]
                    a_dst = _dc.replace(ad, ap=[ad.ap[0], [96, 3], [1152, ni], [1, RES]])
                    a_src = _dc.replace(w3[:], ap=[w3[:].ap[0], [96, 3], [0, ni], [1, RES]])
                    nc.scalar.copy(a_dst, a_src)                      # d_ij W_k
                    # alternate the two HWDGE rings so one transfer's
                    # completion latency overlaps the next one's data
                    eng = nc.sync if (t + i0) % 2 == 0 else nc.scalar
                    eng.dma_start(
                        d_out[128 * t:128 * (t + 1), 864 * i0:864 * (i0 + ni)],
                        O[:, 864 * i0:864 * (i0 + ni)],
                    )

            # first and last tiles stream out in thirds: pulls the first
            # output DMA earlier and drains the tail sooner
            for (b0, b1), tiles in zip(HCHUNKS, CHUNK_TILES):
                emit_chunk(b0, b1)
                for t in tiles:
                    emit_tile(t, split=t in (0, NT - 1))

    nc.finalize()
    return nc


_CACHE = {}


def _get_setup():
    if "nc" not in _CACHE:
        x, DX = _grid_x()
        _CACHE["x"] = x
        _CACHE["static"] = [_build_static(c, x, DX) for c in range(N_CORES)]
        _CACHE["nc"] = _build_program()
    return _CACHE["nc"], _CACHE["static"], _CACHE["x"]


def kernel(BH_positions, BH_masses_presoftplus):
    from concourse.bass_utils import run_bass_kernel_spmd

    nc, static, x = _get_setup()
    pos = np.asarray(BH_positions, np.float64).reshape(2, 3)
    # softplus of the two mass parameters (log1p(exp(x)) in fp32, as jax.nn.softplus)
    pre = np.asarray(BH_masses_presoftplus, np.float32)
    mh = 0.5 * np.log1p(np.exp(pre)).astype(np.float32).astype(np.float64)
    in_maps = [
        {"dmat": static[c]["dmat"], "bb": _build_bb(static[c], x, pos, mh)}
        for c in range(N_CORES)
    ]
    res = run_bass_kernel_spmd(nc, in_maps, list(range(N_CORES)))
    # device tiles are slot-major ([row, s, z]); unshard transposes back
    parts = [
        res.results[c]["out"]
        .reshape(PLANES, RES, S27, RES)
        .transpose(0, 1, 3, 2)
        .reshape(PLANES, RES, RES, 3, 3, 3)
        for c in range(N_CORES)
    ]
    return np.ascontiguousarray(np.concatenate(parts, axis=0))


# revision 22
# speedup vs baseline: 1.3842x; 1.0164x over previous
"""Trainium2 Bass kernel for the Brill-Lindquist Christoffel-symbol grid.

Math: the reference reduces to
    psi  = 1 + sum_n m_n / (2 r_n),   m = softplus(pre)
    h    = psi^4
    G_c  = finite-difference gradient of h along grid axis c (2nd order
           central interior, 1st order one-sided edges, spacing DX)
    W_c  = 0.5 * G_c / h
    Gamma^i_{jk} = delta_ij W_k + delta_ik W_j - delta_jk W_i
so the [96,96,96,3,3,3] output is +-W_c scattered over 27 slots per point.

Sharding: axis 0 (12 planes per core x 8 cores). h is analytic in the
inputs, so each core evaluates its slab plus a 1-plane halo directly --
no inter-core exchange. Per core the grid is row-packed: row = a0*96+a1
(1152 rows -> 9 tiles of 128 partitions), free dim = a2 (96); h lives on
an 11-tile extended row window (halo tiles at both ends).

Axis-0/1 derivatives are bf16 matmuls against per-core FD matrices
(edge one-sided differences folded into the matrix entries); the axis-2
derivative is a shifted free-dim subtract scaled by a per-column vector.
A single bf16 copy of h feeds the matmuls: the rel-err budget (2e-2 of
the global absmax) dwarfs the ~4e-3 it costs.

Output assembly exploits Gamma = delta_ij W_k + delta_ik W_j - delta_jk W_i
directly: each delta-term is ONE strided copy covering 9 slots per point
(stride-0 source broadcast over the free index), with the i=j=k slots
written by all three in a fixed order so the last write (+W_i) is the
correct value. Zero slots persist in reused output buffers.

Schedule: the per-chunk h evaluation is emitted interleaved with the
per-tile output pipeline (chunk0, tile0, chunk1, tiles1-3, ...) so the
1.33 MB/tile output DMAs start within a few us and run back-to-back at
the ~358 GB/s HBM write roofline; all input-derived broadcast rows
(masses/2, (z-pz)^2, (x-px)^2+(y-py)^2 column bases, FD column scale)
are marshaled host-side into one [128, 312] block.
"""

import numpy as np

RES = 96
N_CORES = 8
PLANES = RES // N_CORES        # 12
LROWS = PLANES * RES           # 1152 local rows
NT = LROWS // 128              # 9 local 128-row tiles
EXTNT = NT + 2                 # 11 extended tiles (halo)
NROWS_G = RES * RES            # 9216 global rows
S27 = 27
NOB = 5                        # rotating output buffers

# BB broadcast-block column layout ([128, BBW] fp32, replicated rows)
B_MH = 0        # m1/2, m2/2
B_KV = 2        # z-FD column scale [96]
B_CR = 98       # (z - pz1)^2 [96], (z - pz2)^2 [96]
B_AB = 290      # (x-px)^2+(y-py)^2 column bases [11] x 2
BBW = 312

HCHUNKS = [(0, 3), (3, 6), (6, 9), (9, 11)]   # ext-block ranges
# tiles emitted after each chunk (tile t needs ext blocks t..t+2)
CHUNK_TILES = [(0,), (1, 2, 3), (4, 5, 6), (7, 8)]


def _grid_x():
    # Match the reference grid bit-for-bit: jnp.linspace in fp32 on CPU
    # (the reference's softplus cannot compile for the neuron backend, so
    # it necessarily runs on the jax CPU platform).
    import jax
    import jax.numpy as jnp
    MAX_X = 1.0
    DX = np.float32(MAX_X / (RES / 2 - 1))

    def _ls():
        return jnp.linspace(
            DX * (1 - RES / 2), DX * (RES / 2 - 1), RES, dtype=jnp.float32
        )

    try:
        with jax.default_device(jax.devices("cpu")[0]):
            x = np.asarray(_ls())
    except Exception:
        x = np.asarray(_ls())
    return x, float(DX)


def _fd_sources(idx, coeff_c, coeff_e):
    """(offset, coeff) pairs for d/didx with 1st-order one-sided edges."""
    if idx == 0:
        return [(1, coeff_e), (0, -coeff_e)]
    if idx == RES - 1:
        return [(0, coeff_e), (-1, -coeff_e)]
    return [(1, coeff_c), (-1, -coeff_c)]


def _build_dmat(core, DX):
    """[128, 6*3*128] bf16 FD matrices as matmul lhsT ([q, p] = coeff of
    ext-row q in output row p); 0.5 Christoffel factor folded in. All
    values are +-0.25/DX or +-0.5/DX = +-11.75 / +-23.5, exact in bf16.
    Entries: 0 g0(t=0), 1 g0(interior), 2 g0(t=8), 3..5 g1(t%3)."""
    import ml_dtypes
    c0 = 0.5 * (1.0 / (2.0 * np.float64(DX)))
    ce = 0.5 * (1.0 / np.float64(DX))
    out = np.zeros((128, 6 * 3 * 128), np.float64)

    def fill(entry, t, axis):
        for p in range(128):
            gr = core * LROWS + 128 * t + p
            a = (gr // RES) if axis == 0 else (gr % RES)
            step = RES if axis == 0 else 1
            for off, cf in _fd_sources(a, c0, ce):
                g2 = gr + off * step
                e_ = g2 - core * LROWS + 128
                j = e_ // 128 - t
                q = e_ - 128 * (t + j)
                assert 0 <= j <= 2 and 0 <= q < 128, (core, t, p, off)
                out[q, (entry * 3 + j) * 128 + p] = cf

    fill(0, 0, 0)
    fill(1, 1, 0)
    fill(2, NT - 1, 0)
    for v in range(3):
        fill(3 + v, v, 1)
    return out.astype(ml_dtypes.bfloat16)


def _build_static(core, x, DX):
    slab = core * LROWS
    e = np.arange(EXTNT * 128)
    g = np.clip(slab - 128 + e, 0, NROWS_G - 1)   # clamp halo overrun (unused rows)
    xcol = x[g % RES].reshape(EXTNT, 128).T.copy()     # X coordinate (a1)
    ycol = x[g // RES].reshape(EXTNT, 128).T.copy()    # Y coordinate (a0)
    kvec = np.full(RES, 0.25 / DX, np.float64)
    kvec[0] = kvec[-1] = 0.5 / DX
    return {
        "xcol": xcol.astype(np.float64),
        "ycol": ycol.astype(np.float64),
        "kvec": kvec,
        "dmat": _build_dmat(core, DX),
    }


def _build_bb(static, x, pos, mh):
    """Host-marshaled [128, BBW] broadcast block for one core."""
    bb = np.zeros((128, BBW), np.float64)
    bb[:, B_MH:B_MH + 2] = mh[None, :]
    bb[:, B_KV:B_KV + RES] = static["kvec"][None, :]
    for n in range(2):
        dz = x.astype(np.float64) - pos[n, 2]
        bb[:, B_CR + RES * n:B_CR + RES * (n + 1)] = (dz * dz)[None, :]
        ab = (static["xcol"] - pos[n, 0]) ** 2 + (static["ycol"] - pos[n, 1]) ** 2
        bb[:, B_AB + EXTNT * n:B_AB + EXTNT * (n + 1)] = ab
    return np.ascontiguousarray(bb, np.float32)


def _build_program():
    import dataclasses as _dc

    import concourse.bacc as bacc
    import concourse.mybir as mybir
    import concourse.tile as tile

    DT = mybir.dt.float32
    BF = mybir.dt.bfloat16
    AF = mybir.ActivationFunctionType
    OP = mybir.AluOpType

    nc = bacc.Bacc(None, target_bir_lowering=False, debug=False)
    d_dmat = nc.dram_tensor("dmat", [128, 6 * 3 * 128], BF, kind="ExternalInput")
    d_bb = nc.dram_tensor("bb", [128, BBW], DT, kind="ExternalInput")
    d_out = nc.dram_tensor("out", [LROWS, RES * S27], DT, kind="ExternalOutput")

    HW_ = EXTNT * RES             # 1056: free width of the ext h field
    with tile.TileContext(nc) as tc:
        with (
            tc.tile_pool(name="const", bufs=1) as cpool,
            tc.tile_pool(name="work", bufs=3) as wpool,
            tc.tile_pool(name="wout", bufs=3) as wopool,
            tc.tile_pool(name="obuf", bufs=1) as opool,
            tc.tile_pool(name="psum", bufs=2, space="PSUM") as pspool,
        ):
            # BB gates the whole h pipeline: give it the sync HWDGE ring
            # to itself and put the 4x larger dmat (not needed until the
            # first matmul) on the scalar-engine HWDGE ring
            BBt = cpool.tile([128, BBW], DT)
            nc.sync.dma_start(BBt[:], d_bb[:])
            dm = cpool.tile([128, 6 * 3 * 128], BF)
            nc.scalar.dma_start(dm[:], d_dmat[:])

            H = cpool.tile([128, HW_], DT)
            Hb = cpool.tile([128, HW_], BF)
            otiles = []
            for i in range(NOB):
                O = opool.tile([128, RES * S27], DT, tag=f"ob{i}")
                otiles.append(O)

            def emit_chunk(b0, b1):
                # h = psi^4 on ext blocks [b0, b1); fp32 H + bf16 copy.
                # psi-1 = mh1/r1 + mh2/r2 via the rsqrt activation table
                # (plenty of rel-err budget for its limited precision).
                nb = b1 - b0
                W = nb * RES
                csl = slice(RES * b0, RES * b1)
                qq = []
                for n in range(2):
                    r2 = wpool.tile([128, W], DT, tag="r2")
                    r2v = r2[:].rearrange("p (b z) -> p b z", z=RES)
                    crow = BBt[:, B_CR + RES * n:B_CR + RES * (n + 1)]
                    crow_b = _dc.replace(crow, ap=[crow.ap[0], [0, nb], [1, RES]])
                    absl = BBt[:, B_AB + EXTNT * n + b0:B_AB + EXTNT * n + b1]
                    ab_b = _dc.replace(absl, ap=[absl.ap[0], [1, nb], [0, RES]])
                    nc.gpsimd.tensor_add(r2v[:, :, :], crow_b, ab_b)
                    qn = wpool.tile([128, W], DT, tag=f"q{n}")
                    nc.scalar.activation(qn[:], r2[:], AF.Abs_reciprocal_sqrt)
                    qq.append(qn)
                u2 = wpool.tile([128, W], DT, tag="u2")
                nc.vector.tensor_scalar_mul(u2[:], qq[1][:], BBt[:, B_MH + 1:B_MH + 2])
                psim = wpool.tile([128, W], DT, tag="psim")
                nc.vector.scalar_tensor_tensor(
                    psim[:], qq[0][:], BBt[:, B_MH:B_MH + 1], u2[:], OP.mult, OP.add
                )
                hsq = wpool.tile([128, W], DT, tag="hsq")
                nc.scalar.activation(hsq[:], psim[:], AF.Square, bias=1.0)
                nc.vector.tensor_mul(H[:, csl], hsq[:], hsq[:])
                nc.scalar.activation(Hb[:, csl], hsq[:], AF.Square)

            def emit_tile(t, split):
                O = otiles[t % NOB]
                O3 = O[:].rearrange("p (s z) -> p s z", z=RES)
                if t < NOB:   # one-time zero fill of the 6 zero slots
                    nc.gpsimd.memset(O3[:, 5:8:2, :], 0.0)
                    nc.gpsimd.memset(O3[:, 11:20:4, :], 0.0)
                    nc.gpsimd.memset(O3[:, 21:22, :], 0.0)
                g0e = 0 if t == 0 else (2 if t == NT - 1 else 1)
                g1e = 3 + (t % 3)
                hsl = slice(RES * (t + 1), RES * (t + 2))
                p0 = pspool.tile([128, RES], DT, tag="p0")
                p1 = pspool.tile([128, RES], DT, tag="p1")
                for ge, pp in ((g0e, p0), (g1e, p1)):
                    for j in range(3):
                        lhs = dm[:, (ge * 3 + j) * 128:(ge * 3 + j + 1) * 128]
                        rsl = slice(RES * (t + j), RES * (t + j + 1))
                        nc.tensor.matmul(
                            pp[:], lhs, Hb[:, rsl], start=(j == 0), stop=(j == 2)
                        )

                Hc = H[:, hsl]
                hinv = wopool.tile([128, RES], DT, tag="hinv")
                nc.vector.reciprocal_approx_fast(hinv[:], Hc)
                hz = wopool.tile([128, RES], DT, tag="hz")
                nc.gpsimd.tensor_mul(hz[:], hinv[:], BBt[:, B_KV:B_KV + RES])
                st = wopool.tile([128, RES], DT, tag="st")
                nc.gpsimd.tensor_sub(st[:, 1:95], Hc[:, 2:96], Hc[:, 0:94])
                nc.gpsimd.tensor_sub(st[:, 0:1], Hc[:, 1:2], Hc[:, 0:1])
                nc.gpsimd.tensor_sub(st[:, 95:96], Hc[:, 95:96], Hc[:, 94:95])
                # W planes, slot-major: w3 = [W0 | W1 | W2], 96 each
                w3 = wopool.tile([128, 3 * RES], DT, tag="w3")
                nc.vector.tensor_mul(w3[:, 0:RES], p0[:], hinv[:])
                nc.vector.tensor_mul(w3[:, RES:2 * RES], p1[:], hinv[:])
                nc.vector.tensor_mul(w3[:, 2 * RES:3 * RES], st[:], hz[:])

                # Gamma^i_{jk} = d_ij W_k + d_ik W_j - d_jk W_i: three
                # 9-slot pattern writes (dst element offset 96*slot + z,
                # slots 9i+4j / 10i+3j / 12i+k). The i=j=k slots are
                # written by all three, C -> B -> A; A's +W_i is the
                # correct final value. Each i's slots live in the
                # contiguous third [864*i, 864*(i+1)), so a split tile
                # DMAs each third as soon as its three writes land.
                OW = RES * S27
                for i0, ni in ((0, 1), (1, 1), (2, 1)) if split else ((0, 3),):
                    cd = O[:, 864 * i0:OW]
                    cs = w3[:, RES * i0:3 * RES]
                    c_dst = _dc.replace(cd, ap=[cd.ap[0], [864, ni], [384, 3], [1, RES]])
                    c_src = _dc.replace(cs, ap=[cs.ap[0], [96, ni], [0, 3], [1, RES]])
                    nc.vector.tensor_scalar_mul(c_dst, c_src, -1.0)   # -d_jk W_i
                    bd = O[:, 960 * i0:OW]
                    b_dst = _dc.replace(bd, ap=[bd.ap[0], [288, 3], [960, ni], [1, RES]])
                    b_src = _dc.replace(w3[:], ap=[w3[:].ap[0], [96, 3], [0, ni], [1, RES]])
                    nc.scalar.copy(b_dst, b_src)                      # d_ik W_j
                    ad = O[:, 1152 * i0:OW]
                    a_dst = _dc.replace(ad, ap=[ad.ap[0], [96, 3], [1152, ni], [1, RES]])
                    a_src = _dc.replace(w3[:], ap=[w3[:].ap[0], [96, 3], [0, ni], [1, RES]])
                    nc.scalar.copy(a_dst, a_src)                      # d_ij W_k
                    # alternate the two HWDGE rings so one transfer's
                    # completion latency overlaps the next one's data
                    eng = nc.sync if (t + i0) % 2 == 0 else nc.scalar
                    eng.dma_start(
                        d_out[128 * t:128 * (t + 1), 864 * i0:864 * (i0 + ni)],
                        O[:, 864 * i0:864 * (i0 + ni)],
                    )

            # early and last tiles stream out in thirds: pulls the first
            # output DMAs earlier and drains the tail sooner
            for (b0, b1), tiles in zip(HCHUNKS, CHUNK_TILES):
                emit_chunk(b0, b1)
                for t in tiles:
                    emit_tile(t, split=t in (0, 1, NT - 1))

    nc.finalize()
    return nc


_CACHE = {}


def _get_setup():
    if "nc" not in _CACHE:
        x, DX = _grid_x()
        _CACHE["x"] = x
        _CACHE["static"] = [_build_static(c, x, DX) for c in range(N_CORES)]
        _CACHE["nc"] = _build_program()
    return _CACHE["nc"], _CACHE["static"], _CACHE["x"]


def kernel(BH_positions, BH_masses_presoftplus):
    from concourse.bass_utils import run_bass_kernel_spmd

    nc, static, x = _get_setup()
    pos = np.asarray(BH_positions, np.float64).reshape(2, 3)
    # softplus of the two mass parameters (log1p(exp(x)) in fp32, as jax.nn.softplus)
    pre = np.asarray(BH_masses_presoftplus, np.float32)
    mh = 0.5 * np.log1p(np.exp(pre)).astype(np.float32).astype(np.float64)
    in_maps = [
        {"dmat": static[c]["dmat"], "bb": _build_bb(static[c], x, pos, mh)}
        for c in range(N_CORES)
    ]
    res = run_bass_kernel_spmd(nc, in_maps, list(range(N_CORES)))
    # device tiles are slot-major ([row, s, z]); unshard transposes back
    parts = [
        res.results[c]["out"]
        .reshape(PLANES, RES, S27, RES)
        .transpose(0, 1, 3, 2)
        .reshape(PLANES, RES, RES, 3, 3, 3)
        for c in range(N_CORES)
    ]
    return np.ascontiguousarray(np.concatenate(parts, axis=0))


# revision 25
# speedup vs baseline: 1.3971x; 1.0093x over previous
"""Trainium2 Bass kernel for the Brill-Lindquist Christoffel-symbol grid.

Math: the reference reduces to
    psi  = 1 + sum_n m_n / (2 r_n),   m = softplus(pre)
    h    = psi^4
    G_c  = finite-difference gradient of h along grid axis c (2nd order
           central interior, 1st order one-sided edges, spacing DX)
    W_c  = 0.5 * G_c / h
    Gamma^i_{jk} = delta_ij W_k + delta_ik W_j - delta_jk W_i
so the [96,96,96,3,3,3] output is +-W_c scattered over 27 slots per point.

Sharding: axis 0 (12 planes per core x 8 cores). h is analytic in the
inputs, so each core evaluates its slab plus a 1-plane halo directly --
no inter-core exchange. Per core the grid is row-packed: row = a0*96+a1
(1152 rows -> 9 tiles of 128 partitions), free dim = a2 (96); h lives on
an 11-tile extended row window (halo tiles at both ends).

Axis-0/1 derivatives are bf16 matmuls against per-core FD matrices
(edge one-sided differences folded into the matrix entries); the axis-2
derivative is a shifted free-dim subtract scaled by a per-column vector.
A single bf16 copy of h feeds the matmuls: the rel-err budget (2e-2 of
the global absmax) dwarfs the ~4e-3 it costs.

Output assembly exploits Gamma = delta_ij W_k + delta_ik W_j - delta_jk W_i
directly: each delta-term is ONE strided copy covering 9 slots per point
(stride-0 source broadcast over the free index), with the i=j=k slots
written by all three in a fixed order so the last write (+W_i) is the
correct value. Zero slots persist in reused output buffers.

Schedule: the per-chunk h evaluation is emitted interleaved with the
per-tile output pipeline (chunk0, tile0, chunk1, tiles1-3, ...) so the
1.33 MB/tile output DMAs start within a few us and run back-to-back at
the ~358 GB/s HBM write roofline; all input-derived broadcast rows
(masses/2, (z-pz)^2, (x-px)^2+(y-py)^2 column bases, FD column scale)
are marshaled host-side into one [128, 312] block.
"""

import numpy as np

RES = 96
N_CORES = 8
PLANES = RES // N_CORES        # 12
LROWS = PLANES * RES           # 1152 local rows
NT = LROWS // 128              # 9 local 128-row tiles
EXTNT = NT + 2                 # 11 extended tiles (halo)
NROWS_G = RES * RES            # 9216 global rows
S27 = 27
NOB = 5                        # rotating output buffers

# BB broadcast-block column layout ([128, BBW] fp32, replicated rows)
B_MH = 0        # m1/2, m2/2
B_KV = 2        # z-FD column scale [96]
B_CR = 98       # (z - pz1)^2 [96], (z - pz2)^2 [96]
B_AB = 290      # (x-px)^2+(y-py)^2 column bases [11] x 2
BBW = 312

HCHUNKS = [(0, 3), (3, 6), (6, 9), (9, 11)]   # ext-block ranges
# tiles emitted after each chunk (tile t needs ext blocks t..t+2)
CHUNK_TILES = [(0,), (1, 2, 3), (4, 5, 6), (7, 8)]


def _grid_x():
    # Match the reference grid bit-for-bit: jnp.linspace in fp32 on CPU
    # (the reference's softplus cannot compile for the neuron backend, so
    # it necessarily runs on the jax CPU platform).
    import jax
    import jax.numpy as jnp
    MAX_X = 1.0
    DX = np.float32(MAX_X / (RES / 2 - 1))

    def _ls():
        return jnp.linspace(
            DX * (1 - RES / 2), DX * (RES / 2 - 1), RES, dtype=jnp.float32
        )

    try:
        with jax.default_device(jax.devices("cpu")[0]):
            x = np.asarray(_ls())
    except Exception:
        x = np.asarray(_ls())
    return x, float(DX)


def _fd_sources(idx, coeff_c, coeff_e):
    """(offset, coeff) pairs for d/didx with 1st-order one-sided edges."""
    if idx == 0:
        return [(1, coeff_e), (0, -coeff_e)]
    if idx == RES - 1:
        return [(0, coeff_e), (-1, -coeff_e)]
    return [(1, coeff_c), (-1, -coeff_c)]


def _build_dmat(core, DX):
    """[128, 6*3*128] fp8-e4m3 FD matrices as matmul lhsT ([q, p] = coeff
    of ext-row q in output row p); 0.5 Christoffel factor folded in. The
    exact values +-11.75 / +-23.5 are shipped as the e4m3-exact +-12 /
    +-24 = x(24/23.5); the bf16 h copy is pre-scaled by 23.5/24 (folded
    into its Square activation's scale), so the matmul output is exact.
    Entries: 0 g0(t=0), 1 g0(interior), 2 g0(t=8), 3..5 g1(t%3)."""
    import ml_dtypes
    c0 = 0.5 * (1.0 / (2.0 * np.float64(DX))) * (24.0 / 23.5)
    ce = 0.5 * (1.0 / np.float64(DX)) * (24.0 / 23.5)
    out = np.zeros((128, 6 * 3 * 128), np.float64)

    def fill(entry, t, axis):
        for p in range(128):
            gr = core * LROWS + 128 * t + p
            a = (gr // RES) if axis == 0 else (gr % RES)
            step = RES if axis == 0 else 1
            for off, cf in _fd_sources(a, c0, ce):
                g2 = gr + off * step
                e_ = g2 - core * LROWS + 128
                j = e_ // 128 - t
                q = e_ - 128 * (t + j)
                assert 0 <= j <= 2 and 0 <= q < 128, (core, t, p, off)
                out[q, (entry * 3 + j) * 128 + p] = cf

    fill(0, 0, 0)
    fill(1, 1, 0)
    fill(2, NT - 1, 0)
    for v in range(3):
        fill(3 + v, v, 1)
    return out.astype(ml_dtypes.float8_e4m3)


def _build_static(core, x, DX):
    slab = core * LROWS
    e = np.arange(EXTNT * 128)
    g = np.clip(slab - 128 + e, 0, NROWS_G - 1)   # clamp halo overrun (unused rows)
    xcol = x[g % RES].reshape(EXTNT, 128).T.copy()     # X coordinate (a1)
    ycol = x[g // RES].reshape(EXTNT, 128).T.copy()    # Y coordinate (a0)
    kvec = np.full(RES, 0.25 / DX, np.float64)
    kvec[0] = kvec[-1] = 0.5 / DX
    return {
        "xcol": xcol.astype(np.float64),
        "ycol": ycol.astype(np.float64),
        "kvec": kvec,
        "dmat": _build_dmat(core, DX),
    }


def _build_bb(static, x, pos, mh):
    """Host-marshaled [128, BBW] broadcast block for one core."""
    bb = np.zeros((128, BBW), np.float64)
    bb[:, B_MH:B_MH + 2] = mh[None, :]
    bb[:, B_KV:B_KV + RES] = static["kvec"][None, :]
    for n in range(2):
        dz = x.astype(np.float64) - pos[n, 2]
        bb[:, B_CR + RES * n:B_CR + RES * (n + 1)] = (dz * dz)[None, :]
        ab = (static["xcol"] - pos[n, 0]) ** 2 + (static["ycol"] - pos[n, 1]) ** 2
        bb[:, B_AB + EXTNT * n:B_AB + EXTNT * (n + 1)] = ab
    return np.ascontiguousarray(bb, np.float32)


def _build_program():
    import dataclasses as _dc

    import concourse.bacc as bacc
    import concourse.mybir as mybir
    import concourse.tile as tile

    DT = mybir.dt.float32
    BF = mybir.dt.bfloat16
    F8 = mybir.dt.float8e4
    AF = mybir.ActivationFunctionType
    OP = mybir.AluOpType

    nc = bacc.Bacc(None, target_bir_lowering=False, debug=False)
    d_dmat = nc.dram_tensor("dmat", [128, 6 * 3 * 128], F8, kind="ExternalInput")
    d_bb = nc.dram_tensor("bb", [128, BBW], DT, kind="ExternalInput")
    d_out = nc.dram_tensor("out", [LROWS, RES * S27], DT, kind="ExternalOutput")

    HW_ = EXTNT * RES             # 1056: free width of the ext h field
    with tile.TileContext(nc) as tc:
        with (
            tc.tile_pool(name="const", bufs=1) as cpool,
            tc.tile_pool(name="work", bufs=3) as wpool,
            tc.tile_pool(name="wout", bufs=3) as wopool,
            tc.tile_pool(name="obuf", bufs=1) as opool,
            tc.tile_pool(name="psum", bufs=2, space="PSUM") as pspool,
        ):
            # BB gates the whole h pipeline: give it the sync HWDGE ring
            # to itself and put the 4x larger dmat (not needed until the
            # first matmul) on the scalar-engine HWDGE ring
            BBt = cpool.tile([128, BBW], DT)
            nc.sync.dma_start(BBt[:], d_bb[:])
            dm = cpool.tile([128, 6 * 3 * 128], F8)
            nc.scalar.dma_start(dm[:], d_dmat[:])

            H = cpool.tile([128, HW_], DT)
            Hb = cpool.tile([128, HW_], BF)
            otiles = []
            for i in range(NOB):
                O = opool.tile([128, RES * S27], DT, tag=f"ob{i}")
                otiles.append(O)

            def emit_chunk(b0, b1):
                # h = psi^4 on ext blocks [b0, b1); fp32 H + bf16 copy.
                # psi-1 = mh1/r1 + mh2/r2 via the rsqrt activation table
                # (plenty of rel-err budget for its limited precision).
                nb = b1 - b0
                W = nb * RES
                csl = slice(RES * b0, RES * b1)
                qq = []
                for n in range(2):
                    r2 = wpool.tile([128, W], DT, tag="r2")
                    r2v = r2[:].rearrange("p (b z) -> p b z", z=RES)
                    crow = BBt[:, B_CR + RES * n:B_CR + RES * (n + 1)]
                    crow_b = _dc.replace(crow, ap=[crow.ap[0], [0, nb], [1, RES]])
                    absl = BBt[:, B_AB + EXTNT * n + b0:B_AB + EXTNT * n + b1]
                    ab_b = _dc.replace(absl, ap=[absl.ap[0], [1, nb], [0, RES]])
                    nc.gpsimd.tensor_add(r2v[:, :, :], crow_b, ab_b)
                    qn = wpool.tile([128, W], DT, tag=f"q{n}")
                    nc.scalar.activation(qn[:], r2[:], AF.Abs_reciprocal_sqrt)
                    qq.append(qn)
                u2 = wpool.tile([128, W], DT, tag="u2")
                nc.vector.tensor_scalar_mul(u2[:], qq[1][:], BBt[:, B_MH + 1:B_MH + 2])
                psim = wpool.tile([128, W], DT, tag="psim")
                nc.vector.scalar_tensor_tensor(
                    psim[:], qq[0][:], BBt[:, B_MH:B_MH + 1], u2[:], OP.mult, OP.add
                )
                hsq = wpool.tile([128, W], DT, tag="hsq")
                nc.scalar.activation(hsq[:], psim[:], AF.Square, bias=1.0)
                nc.vector.tensor_mul(H[:, csl], hsq[:], hsq[:])
                # Hb = h * 23.5/24, cancelling the fp8 coefficient rescale
                nc.scalar.activation(Hb[:, csl], hsq[:], AF.Square, scale=float(np.sqrt(23.5 / 24.0)))

            def emit_tile(t, split):
                O = otiles[t % NOB]
                O3 = O[:].rearrange("p (s z) -> p s z", z=RES)
                if t < NOB:   # one-time zero fill of the 6 zero slots
                    nc.gpsimd.memset(O3[:, 5:8:2, :], 0.0)
                    nc.gpsimd.memset(O3[:, 11:20:4, :], 0.0)
                    nc.gpsimd.memset(O3[:, 21:22, :], 0.0)
                g0e = 0 if t == 0 else (2 if t == NT - 1 else 1)
                g1e = 3 + (t % 3)
                hsl = slice(RES * (t + 1), RES * (t + 2))
                p0 = pspool.tile([128, RES], DT, tag="p0")
                p1 = pspool.tile([128, RES], DT, tag="p1")
                for ge, pp in ((g0e, p0), (g1e, p1)):
                    for j in range(3):
                        lhs = dm[:, (ge * 3 + j) * 128:(ge * 3 + j + 1) * 128]
                        rsl = slice(RES * (t + j), RES * (t + j + 1))
                        nc.tensor.matmul(
                            pp[:], lhs, Hb[:, rsl], start=(j == 0), stop=(j == 2)
                        )

                Hc = H[:, hsl]
                hinv = wopool.tile([128, RES], DT, tag="hinv")
                nc.vector.reciprocal_approx_fast(hinv[:], Hc)
                hz = wopool.tile([128, RES], DT, tag="hz")
                nc.gpsimd.tensor_mul(hz[:], hinv[:], BBt[:, B_KV:B_KV + RES])
                st = wopool.tile([128, RES], DT, tag="st")
                nc.gpsimd.tensor_sub(st[:, 1:95], Hc[:, 2:96], Hc[:, 0:94])
                nc.gpsimd.tensor_sub(st[:, 0:1], Hc[:, 1:2], Hc[:, 0:1])
                nc.gpsimd.tensor_sub(st[:, 95:96], Hc[:, 95:96], Hc[:, 94:95])
                # W planes, slot-major: w3 = [W0 | W1 | W2], 96 each
                w3 = wopool.tile([128, 3 * RES], DT, tag="w3")
                nc.vector.tensor_mul(w3[:, 0:RES], p0[:], hinv[:])
                nc.vector.tensor_mul(w3[:, RES:2 * RES], p1[:], hinv[:])
                nc.vector.tensor_mul(w3[:, 2 * RES:3 * RES], st[:], hz[:])

                # Gamma^i_{jk} = d_ij W_k + d_ik W_j - d_jk W_i: three
                # 9-slot pattern writes (dst element offset 96*slot + z,
                # slots 9i+4j / 10i+3j / 12i+k). The i=j=k slots are
                # written by all three, C -> B -> A; A's +W_i is the
                # correct final value. Each i's slots live in the
                # contiguous third [864*i, 864*(i+1)), so a split tile
                # DMAs each third as soon as its three writes land.
                OW = RES * S27
                # C skips the i=j=k slots (per-i two-slot ops, disjoint
                # from B and A), so only B -> A serialize on those slots
                C_SLOTS = ((384, 384), (864, 768), (1728, 384))
                for i0, ni in ((0, 1), (1, 1), (2, 1)) if split else ((0, 3),):
                    for i in range(i0, i0 + ni):
                        base, stride = C_SLOTS[i]
                        cd = O[:, base:OW]
                        cs = w3[:, RES * i:3 * RES]
                        c_dst = _dc.replace(cd, ap=[cd.ap[0], [1, RES], [stride, 2]])
                        c_src = _dc.replace(cs, ap=[cs.ap[0], [1, RES], [0, 2]])
                        nc.vector.tensor_scalar_mul(c_dst, c_src, -1.0)  # -d_jk W_i
                    bd = O[:, 960 * i0:OW]
                    b_dst = _dc.replace(bd, ap=[bd.ap[0], [288, 3], [960, ni], [1, RES]])
                    b_src = _dc.replace(w3[:], ap=[w3[:].ap[0], [96, 3], [0, ni], [1, RES]])
                    nc.scalar.copy(b_dst, b_src)                      # d_ik W_j
                    ad = O[:, 1152 * i0:OW]
                    a_dst = _dc.replace(ad, ap=[ad.ap[0], [96, 3], [1152, ni], [1, RES]])
                    a_src = _dc.replace(w3[:], ap=[w3[:].ap[0], [96, 3], [0, ni], [1, RES]])
                    nc.scalar.copy(a_dst, a_src)                      # d_ij W_k
                    # alternate the two HWDGE rings so one transfer's
                    # completion latency overlaps the next one's data
                    eng = nc.sync if (t + i0) % 2 == 0 else nc.scalar
                    eng.dma_start(
                        d_out[128 * t:128 * (t + 1), 864 * i0:864 * (i0 + ni)],
                        O[:, 864 * i0:864 * (i0 + ni)],
                    )

            # early and last tiles stream out in thirds: pulls the first
            # output DMAs earlier and drains the tail sooner
            for (b0, b1), tiles in zip(HCHUNKS, CHUNK_TILES):
                emit_chunk(b0, b1)
                for t in tiles:
                    emit_tile(t, split=t in (0, 1, NT - 1))

    nc.finalize()
    return nc


_CACHE = {}


def _get_setup():
    if "nc" not in _CACHE:
        x, DX = _grid_x()
        _CACHE["x"] = x
        _CACHE["static"] = [_build_static(c, x, DX) for c in range(N_CORES)]
        _CACHE["nc"] = _build_program()
    return _CACHE["nc"], _CACHE["static"], _CACHE["x"]


def kernel(BH_positions, BH_masses_presoftplus):
    from concourse.bass_utils import run_bass_kernel_spmd

    nc, static, x = _get_setup()
    pos = np.asarray(BH_positions, np.float64).reshape(2, 3)
    # softplus of the two mass parameters (log1p(exp(x)) in fp32, as jax.nn.softplus)
    pre = np.asarray(BH_masses_presoftplus, np.float32)
    mh = 0.5 * np.log1p(np.exp(pre)).astype(np.float32).astype(np.float64)
    in_maps = [
        {"dmat": static[c]["dmat"], "bb": _build_bb(static[c], x, pos, mh)}
        for c in range(N_CORES)
    ]
    res = run_bass_kernel_spmd(nc, in_maps, list(range(N_CORES)))
    # device tiles are slot-major ([row, s, z]); unshard transposes back
    parts = [
        res.results[c]["out"]
        .reshape(PLANES, RES, S27, RES)
        .transpose(0, 1, 3, 2)
        .reshape(PLANES, RES, RES, 3, 3, 3)
        for c in range(N_CORES)
    ]
    return np.ascontiguousarray(np.concatenate(parts, axis=0))
